# revision 18
# baseline (speedup 1.0000x reference)
"""A3C network (4-layer MLP + LSTMCell + 3 heads), batch=1, on 8 TRN2 NeuronCores.

Strategy: the network is ~274K params (~1.1MB f32) with batch=1 — memory-bound,
no useful sharding (per the problem hint). We replicate the whole forward pass
on every core (A3C-style); core 0's output is returned. Inside each core the
run time is dominated by the HBM->SBUF weight DMA, so:
  - weights are pre-packed on the host into THREE contiguous [128, C] fp16
    tensors (pre-transposed into the TensorEngine's lhsT layout), halving DMA
    bytes vs f32 and allowing maximal DMA coalescing;
  - the three DMAs are pipelined with compute (layer k computes while the
    weights for layer k+1 stream in);
  - biases are fused into the ScalarEngine activation ops (func(in+bias));
  - when hx / cx / all biases are zero (the declared input distribution:
    fill=zeros), the builder specializes: the w_hh matmuls, cx term, and all
    bias handling are dropped, cutting DMA bytes to ~430KB.
"""

import os
import sys

import numpy as np

for _p in ("/opt/trn_rl_repo", os.path.expanduser("~/.axon_site/_ro/trn_rl_repo")):
    if os.path.isdir(_p) and _p not in sys.path:
        sys.path.insert(0, _p)

import concourse.bass as bass
import concourse.mybir as mybir
from concourse.bass_utils import run_bass_kernel_spmd

F16 = mybir.dt.float16
F32 = mybir.dt.float32

STATE = 29
H1 = 256  # layer1/2 width
H2 = 128  # layer3/4 + LSTM width
NOUT = 7  # value(1) + actor(3) + actor2(3)


def _packs(inputs):
    """Host-side prep: transpose weights into lhsT layout and pack into three
    contiguous [128, C] fp16 arrays (+ optional [128,12] f32 bias pack)."""
    f32 = np.float32
    x = np.asarray(inputs["x"], f32).reshape(-1)
    hx = np.asarray(inputs["hx"], f32).reshape(-1)
    cx = np.asarray(inputs["cx"], f32).reshape(-1)
    w1 = np.asarray(inputs["w1"], f32)
    w2 = np.asarray(inputs["w2"], f32)
    w3 = np.asarray(inputs["w3"], f32)
    w4 = np.asarray(inputs["w4"], f32)
    w_ih = np.asarray(inputs["w_ih"], f32)
    w_hh = np.asarray(inputs["w_hh"], f32)
    # head row order [actor(3), value(1), actor2(3)] so the softsign rows sit
    # at partition base 0 (compute engines require aligned partition bases)
    heads = np.concatenate(
        [np.asarray(inputs[k], f32) for k in ("w_actor", "w_critic", "w_actor2")], 0
    )  # [7, 128]

    biases = {k: np.asarray(inputs[k], f32) for k in
              ("b1", "b2", "b3", "b4", "b_ih", "b_hh", "b_critic", "b_actor", "b_actor2")}
    cfg = {
        "use_hh": bool(np.any(hx)),
        "use_cx": bool(np.any(cx)),
        "use_bias": any(bool(np.any(v)) for v in biases.values()),
    }

    def col(v, n=128):
        c = np.zeros((128, 1), np.float16)
        c[: len(v), 0] = v.astype(np.float16)
        return c

    # gate order in psum: [i, f, o, g] so sigmoid covers a contiguous 3-col run
    GATES = (0, 1, 3, 2)

    # --- w_a: x [, hx, cx], w1T, [w_hhT] ---
    parts_a = [col(x)]
    off_a = {"x": 0}
    n = 1
    if cfg["use_hh"]:
        off_a["hx"] = n
        parts_a.append(col(hx))
        n += 1
    if cfg["use_cx"]:
        off_a["cx"] = n
        parts_a.append(col(cx))
        n += 1
    off_a["w1"] = n
    w1t = np.zeros((128, H1), np.float16)
    w1t[:STATE, :] = w1.T.astype(np.float16)
    parts_a.append(w1t)
    n += H1
    if cfg["use_hh"]:
        off_a["whh"] = n
        whht = w_hh.T.astype(np.float16)  # [128, 512]
        parts_a.append(np.concatenate([whht[:, g * 128:(g + 1) * 128] for g in GATES], 1))
        n += 512
    if n % 2:
        parts_a.append(np.zeros((128, 1), np.float16))
        n += 1
    w_a = np.ascontiguousarray(np.concatenate(parts_a, 1))

    # --- w_b: w2T blocks (m-major), w3T k-blocks, w4T ---
    w2t = w2.T.astype(np.float16)  # [256, 256]
    blocks = [w2t[k * 128:(k + 1) * 128, m * 128:(m + 1) * 128]
              for m in (0, 1) for k in (0, 1)]
    w3t = w3.T.astype(np.float16)  # [256, 128]
    blocks += [w3t[k * 128:(k + 1) * 128, :] for k in (0, 1)]
    blocks.append(w4.T.astype(np.float16))
    off_b = {"w2": 0, "w3": 512, "w4": 768}
    w_b = np.ascontiguousarray(np.concatenate(blocks, 1))  # [128, 896]

    # --- w_c: w_ihT (gate order i,f,o,g), headsT ---
    wiht = w_ih.T.astype(np.float16)  # [128, 512]
    parts_c = [wiht[:, g * 128:(g + 1) * 128] for g in GATES]
    parts_c.append(heads.T.astype(np.float16))  # [128, 7]
    parts_c.append(np.zeros((128, 1), np.float16))
    off_c = {"wih": 0, "heads": 512}
    w_c = np.ascontiguousarray(np.concatenate(parts_c, 1))  # [128, 520]

    arrays = {"w_a": w_a, "w_b": w_b, "w_c": w_c}
    cfg["ca"], cfg["cb"], cfg["cc"] = w_a.shape[1], w_b.shape[1], w_c.shape[1]
    cfg["off_a"], cfg["off_b"], cfg["off_c"] = off_a, off_b, off_c

    if cfg["use_bias"]:
        bg = biases["b_ih"] + biases["b_hh"]  # [512]
        bp = np.zeros((128, 18), np.float32)
        bp[:, 0] = biases["b1"][:128]
        bp[:, 1] = biases["b1"][128:]
        bp[:, 2] = biases["b2"][:128]
        bp[:, 3] = biases["b2"][128:]
        bp[:, 4] = biases["b3"]
        bp[:, 5] = biases["b4"]
        for t, g in enumerate(GATES):  # cols 6..9 = gates i,f,o,g
            bp[:, 6 + t] = bg[g * 128:(g + 1) * 128]
        bp[0:3, 10] = biases["b_actor"]
        bp[3, 10] = biases["b_critic"][0]
        bp[4:7, 10] = biases["b_actor2"]
        bp[:, 12:18] = 0.1 * bp[:, 0:6]  # pre-scaled MLP biases for the 0.1v path
        arrays["bias"] = bp

    return cfg, arrays


def _build(cfg):
    use_hh, use_cx, use_bias = cfg["use_hh"], cfg["use_cx"], cfg["use_bias"]
    oa, ob, oc = cfg["off_a"], cfg["off_b"], cfg["off_c"]
    AF = mybir.ActivationFunctionType
    ALU = mybir.AluOpType
    SLOPE = 0.1  # leaky relu: max(v, SLOPE*v) on DVE (HW Lrelu alpha is fixed)

    nc = bass.Bass(trn_type="TRN2", debug=False)
    d_bias = None
    if use_bias:
        d_bias = nc.declare_dram_parameter("bias", [128, 12], F32, isOutput=False)
    d_wa = nc.declare_dram_parameter("w_a", [128, cfg["ca"]], F16, isOutput=False)
    d_wb = nc.declare_dram_parameter("w_b", [128, cfg["cb"]], F16, isOutput=False)
    d_wc = nc.declare_dram_parameter("w_c", [128, cfg["cc"]], F16, isOutput=False)
    d_out = nc.declare_dram_parameter("out", [NOUT, 1], F32, isOutput=True)

    # dma_sem thresholds (each dma_start adds 16)
    nb = 16 if use_bias else 0
    dma_wa, dma_wb, dma_wc = nb + 16, nb + 32, nb + 48
    dma_out = nb + 64

    # --- op numbering per engine (must match emission order below) ---
    MLP = ["m1a", "m1b", "m2a", "m2b", "m3", "m4"]
    pe = dict(l1a=1, l1b=2, l2a=3, l2b=4, l3=5, l4=6, gi=7, gf=8, go=9, gg=10,
              heads=11)
    act, dve = {}, {}
    an = dn = 0
    for m in MLP:
        if use_bias:  # u = v+b, t = 0.1v+0.1b on ACT
            an += 1
            act["u" + m] = an
        an += 1  # no-bias: t = 0.1*v on ACT
        act["t" + m] = an
    for m in MLP:  # DVE leaky-relu max
        dn += 1
        dve[m] = dn
    if use_bias:
        for g in ("si", "sf", "so", "tg"):
            an += 1
            act[g] = an
    else:
        an += 1
        act["si"] = act["sf"] = act["so"] = an  # one sigmoid over [i,f,o]
        an += 1
        act["tg"] = an
    if use_cx:
        dn += 1
        dve["sitg"] = dn
    dn += 1
    dve["cnew"] = dn
    an += 1
    act["th"] = an
    dn += 1
    dve["hnew"] = dn
    an += 1
    act["oraw"] = an
    an += 1
    act["abs"] = an
    dn += 1
    dve["denom"] = dn
    dn += 1
    dve["recip"] = dn
    dn += 1
    dve["actor"] = dn

    # sh cols (fp16 activations): h1a h1b h2a h2b h3 h4 hnew
    C_H1A, C_H1B, C_H2A, C_H2B, C_H3, C_H4, C_HNEW = range(7)
    # scr cols (f32): t0..5 (=0.1v), u0..5 (=v+b, bias path), then LSTM
    T0, U0 = 0, 6
    S_SI, S_SF, S_SO, S_TG, S_CNEW, S_TH, S_ABS = 12, 13, 14, 15, 16, 17, 18
    # ps_mlp cols: h1a h1b h2a h2b h3 h4 heads
    P_HEADS = 6

    with (
        nc.sbuf_tensor("sa", [128, cfg["ca"]], F16) as sa,
        nc.sbuf_tensor("sb", [128, cfg["cb"]], F16) as sb,
        nc.sbuf_tensor("sc", [128, cfg["cc"]], F16) as sc,
        nc.sbuf_tensor("sh", [128, 8], F16) as sh,
        nc.sbuf_tensor("scr", [128, 20], F32) as scr,
        nc.sbuf_tensor("osb", [128, 1], F32) as osb,
        (nc.sbuf_tensor("sbias", [128, 18], F32) if use_bias else _null_ctx()) as sbias,
        nc.psum_tensor("ps_mlp", [128, 8], F32) as ps,
        nc.psum_tensor("ps_g", [128, 4], F32) as psg,
        nc.semaphore("dma_sem") as dma_sem,
        nc.semaphore("pe_sem") as pe_sem,
        nc.semaphore("act_sem") as act_sem,
        nc.semaphore("dve_sem") as dve_sem,
        nc.Block() as block,
    ):
        @block.sync
        def _(sync):
            if use_bias:
                sync.dma_start(out=sbias[:, :], in_=d_bias[:, :]).then_inc(dma_sem, 16)
            sync.dma_start(out=sa[:, :], in_=d_wa[:, :]).then_inc(dma_sem, 16)
            sync.dma_start(out=sb[:, :], in_=d_wb[:, :]).then_inc(dma_sem, 16)
            sync.dma_start(out=sc[:, :], in_=d_wc[:, :]).then_inc(dma_sem, 16)
            sync.wait_ge(dve_sem, dve["actor"])
            sync.dma_start(out=d_out[:, :], in_=osb[0:NOUT, 0:1]).then_inc(dma_sem, 16)
            sync.wait_ge(dma_sem, dma_out)
            # reset all semaphores so the NEFF is re-entrant (profiling and
            # benchmarking re-execute the same loaded NEFF)
            for s in (dma_sem, pe_sem, act_sem, dve_sem):
                sync.sem_clear(s)

        @block.tensor
        def _(tensor):
            x_col = sa[:, oa["x"]:oa["x"] + 1]
            tensor.wait_ge(dma_sem, dma_wa)
            if use_hh:  # open the 4 gate accumulation groups with the hx term
                hx_col = sa[:, oa["hx"]:oa["hx"] + 1]
                for t in range(4):
                    c = oa["whh"] + t * 128
                    tensor.matmul(psg[:, t:t + 1], sa[:, c:c + 128], hx_col,
                                  start=True, stop=False)
            # L1: [29->256] as two 128-wide halves
            c = oa["w1"]
            tensor.matmul(ps[:, 0:1], sa[:, c:c + 128], x_col).then_inc(pe_sem, 1)
            tensor.matmul(ps[:, 1:2], sa[:, c + 128:c + 256], x_col).then_inc(pe_sem, 1)
            # L2: [256->256], 2 output halves x 2 k-blocks
            tensor.wait_ge(dma_sem, dma_wb)
            tensor.wait_ge(dve_sem, dve["m1b"])
            for m in range(2):
                for kk in range(2):
                    c = ob["w2"] + (m * 2 + kk) * 128
                    mm = tensor.matmul(ps[:, 2 + m:3 + m], sb[:, c:c + 128],
                                       sh[:, C_H1A + kk:C_H1A + kk + 1],
                                       start=(kk == 0), stop=(kk == 1))
                    if kk == 1:
                        mm.then_inc(pe_sem, 1)
            # L3: [256->128]
            tensor.wait_ge(dve_sem, dve["m2b"])
            for kk in range(2):
                c = ob["w3"] + kk * 128
                mm = tensor.matmul(ps[:, 4:5], sb[:, c:c + 128],
                                   sh[:, C_H2A + kk:C_H2A + kk + 1],
                                   start=(kk == 0), stop=(kk == 1))
            mm.then_inc(pe_sem, 1)
            # L4: [128->128]
            tensor.wait_ge(dve_sem, dve["m3"])
            c = ob["w4"]
            tensor.matmul(ps[:, 5:6], sb[:, c:c + 128],
                          sh[:, C_H3:C_H3 + 1]).then_inc(pe_sem, 1)
            # gates: w_ih @ h4 (+ open hh group), order i,f,o,g
            tensor.wait_ge(dma_sem, dma_wc)
            tensor.wait_ge(dve_sem, dve["m4"])
            h4_col = sh[:, C_H4:C_H4 + 1]
            for t in range(4):
                c = oc["wih"] + t * 128
                tensor.matmul(psg[:, t:t + 1], sc[:, c:c + 128], h4_col,
                              start=not use_hh, stop=True).then_inc(pe_sem, 1)
            # heads: [128->7]
            tensor.wait_ge(dve_sem, dve["hnew"])
            c = oc["heads"]
            tensor.matmul(ps[0:NOUT, P_HEADS:P_HEADS + 1], sc[:, c:c + NOUT],
                          sh[:, C_HNEW:C_HNEW + 1]).then_inc(pe_sem, 1)

        @block.scalar
        def _(scalar):
            if use_bias:
                scalar.wait_ge(dma_sem, 16)
                for i, m in enumerate(MLP):  # u = v+b ; t = 0.1v + 0.1b
                    scalar.wait_ge(pe_sem, i + 1)
                    scalar.activation(scr[:, U0 + i:U0 + i + 1], ps[:, i:i + 1],
                                      AF.Identity,
                                      bias=sbias[:, i:i + 1]).then_inc(act_sem, 1)
                    scalar.activation(scr[:, T0 + i:T0 + i + 1], ps[:, i:i + 1],
                                      AF.Identity, scale=0.1,
                                      bias=sbias[:, 12 + i:13 + i]).then_inc(act_sem, 1)
                for t, (g, fn) in enumerate([("si", AF.Sigmoid), ("sf", AF.Sigmoid),
                                             ("so", AF.Sigmoid), ("tg", AF.Tanh)]):
                    # psg col order is i,f,o,g; bias cols 6..9 follow it
                    scol = {"si": S_SI, "sf": S_SF, "so": S_SO, "tg": S_TG}[g]
                    scalar.wait_ge(pe_sem, pe[("gi", "gf", "go", "gg")[t]])
                    scalar.activation(scr[:, scol:scol + 1], psg[:, t:t + 1], fn,
                                      bias=sbias[:, 6 + t:7 + t]).then_inc(act_sem, 1)
            else:
                for i, m in enumerate(MLP):  # t = 0.1*v
                    scalar.wait_ge(pe_sem, i + 1)
                    scalar.activation(scr[:, T0 + i:T0 + i + 1], ps[:, i:i + 1],
                                      AF.Identity, scale=0.1).then_inc(act_sem, 1)
            if not use_bias:
                scalar.wait_ge(pe_sem, pe["go"])
                scalar.activation(scr[:, S_SI:S_SI + 3], psg[:, 0:3],
                                  AF.Sigmoid).then_inc(act_sem, 1)
                scalar.wait_ge(pe_sem, pe["gg"])
                scalar.activation(scr[:, S_TG:S_TG + 1], psg[:, 3:4],
                                  AF.Tanh).then_inc(act_sem, 1)
            # th = tanh(c_new)
            scalar.wait_ge(dve_sem, dve["cnew"])
            scalar.activation(scr[:, S_TH:S_TH + 1], scr[:, S_CNEW:S_CNEW + 1],
                              AF.Tanh).then_inc(act_sem, 1)
            # heads epilogue: raw = psum + bias; abs for softsign
            scalar.wait_ge(pe_sem, pe["heads"])
            scalar.activation(osb[0:NOUT, 0:1], ps[0:NOUT, P_HEADS:P_HEADS + 1],
                              AF.Identity,
                              bias=(sbias[0:NOUT, 10:11] if use_bias else 0.0)
                              ).then_inc(act_sem, 1)
            scalar.activation(scr[0:3, S_ABS:S_ABS + 1], ps[0:3, P_HEADS:P_HEADS + 1],
                              AF.Abs,
                              bias=(sbias[0:3, 10:11] if use_bias else 0.0)
                              ).then_inc(act_sem, 1)

        @block.vector
        def _(vector):
            # leaky relu: h = max(v+b, 0.1(v+b)); t (and u in bias path) from ACT
            for i, m in enumerate(MLP):
                vector.wait_ge(act_sem, act["t" + m])
                t = scr[:, T0 + i:T0 + i + 1]
                u = scr[:, U0 + i:U0 + i + 1] if use_bias else ps[:, i:i + 1]
                vector.tensor_max(sh[:, i:i + 1], u, t).then_inc(dve_sem, 1)
            si = scr[:, S_SI:S_SI + 1]
            sf = scr[:, S_SF:S_SF + 1]
            so = scr[:, S_SO:S_SO + 1]
            tg = scr[:, S_TG:S_TG + 1]
            cnew = scr[:, S_CNEW:S_CNEW + 1]
            vector.wait_ge(act_sem, act["tg"])
            if use_cx:
                sitg = scr[:, S_ABS:S_ABS + 1]  # abs col is free until later
                vector.tensor_mul(sitg, si, tg).then_inc(dve_sem, 1)
                cx_col = sa[:, oa["cx"]:oa["cx"] + 1]
                vector.scalar_tensor_tensor(
                    cnew, cx_col, sf, sitg,
                    op0=ALU.mult, op1=ALU.add).then_inc(dve_sem, 1)
            else:
                vector.tensor_mul(cnew, si, tg).then_inc(dve_sem, 1)
            vector.wait_ge(act_sem, act["th"])
            vector.tensor_mul(sh[:, C_HNEW:C_HNEW + 1], scr[:, S_TH:S_TH + 1],
                              so).then_inc(dve_sem, 1)
            # softsign on actor rows 0..2: o[0:3] *= 1/(1+|raw|)
            ab = scr[0:3, S_ABS:S_ABS + 1]
            vector.wait_ge(act_sem, act["abs"])
            vector.tensor_scalar_add(ab, ab, 1.0).then_inc(dve_sem, 1)
            vector.reciprocal(ab, ab).then_inc(dve_sem, 1)
            vector.tensor_mul(osb[0:3, 0:1], osb[0:3, 0:1], ab).then_inc(dve_sem, 1)

    return nc


class _null_ctx:
    def __enter__(self):
        return None

    def __exit__(self, *a):
        return False


def _run(inputs, trace=False, **kw):
    cfg, arrays = _packs(inputs)
    nc = _build(cfg)
    in_maps = [arrays] * 8
    res = run_bass_kernel_spmd(nc, in_maps, core_ids=list(range(8)), trace=trace, **kw)
    out = np.asarray(res.results[0]["out"], np.float32).reshape(-1)
    actor = out[0:3].copy()
    value = np.asarray(out[3], np.float32)
    actor2 = out[4:7].copy()
    return (value, actor, actor2), res


def kernel(**inputs):
    outs, _ = _run(inputs, trace=False)
    return outs
